# revision 12
# baseline (speedup 1.0000x reference)
"""Trainium2 Bass kernel for nn_DetectionLoss (2-class detection loss).

Computes, over B=2^24 rows of logits [B,2] and labels [B]:
  ce    = mean(-log_softmax(outputs)[label])
  pred  = argmax(outputs, axis=1)
  confusion counts TP/TN/FP/FN from (label, pred)
  CS    = M[pred, label] with M = [[0,1],[0,0]]  -> mean(CS) = FN/B
  loss  = ce + coeff(TP,TN,FP,FN) * mean(CS)

Device math (2 classes): with d = x1 - x0 and h = label - 0.5:
  u       = d*h                  # sign-folded logit margin
  ce_row  = softplus(-2u) = log(1 + exp(-2u))
  pred    = (d > 0)
  correct = (u > 0)              # prediction == label
Counts follow from three linear sums (n1 = sum(h) + B/2, p1 = sum(pred),
TP + TN = sum(correct)):
  TP = (sum(correct) + p1 + n1 - B) / 2, TN = sum(correct) - TP,
  FP = p1 - TP, FN = n1 - TP.

Engine split per chunk — every reduction rides on an op via the
DVE/ACT fused per-partition accumulator (accum_out) or a GpSimd
cross-partition reduce, so the tensor engine and PSUM are not used:
  DVE: d = x1 - x0;  u = (lab - 0.5)*d  [scalar_tensor_tensor];
       pred = d > 0 with accum -> sum(pred)
  GPS: tensor_reduce(lab) -> sum(labels) = n1 (per chunk scalar)
  ACT: t = exp(-2u); ln(1+t) with accum -> CE partial;
       Sign(u) with accum -> sum(sign(u)) = 2*sum(correct) - count
Per 2048-row chunk each engine is busy well under the ~7.4 us the two
input DMAs take, so the kernel streams at DMA line rate. Inputs flow
through SBUF in variable-size chunks (small at both ends to shorten
pipeline fill/drain). The tiny per-core accumulator tile is combined on
the host; count arithmetic is exact (half-integers in fp32).

Sharding: data-parallel over the batch dim across 8 NeuronCores.
"""

import numpy as np

import concourse.bass as bass
import concourse.mybir as mybir
import concourse.tile as tile
from concourse.bass_utils import run_bass_kernel_spmd

N_CORES = 8
P = 128
LAMBD = 0.5

_cache = {}

_MAX_WAITS = 1  # this walrus build rejects >1 embedded sync-wait per instruction


def _split_multiwaits(nc):
    """Walrus in this container can't encode instructions with multiple
    sync waits; hoist all but the last into standalone EventSemaphore
    waits on the same engine immediately before the instruction."""
    n = [0]

    def fix_block(blk):
        new_insts = []
        for ins in blk.instructions:
            si = ins.sync_info
            if si is not None and si.on_wait and len(si.on_wait) > _MAX_WAITS:
                waits = list(si.on_wait)
                for w in waits[: -_MAX_WAITS]:
                    n[0] += 1
                    ev = mybir.InstEventSemaphore(
                        name=f"I-waitsplit-{n[0]}",
                        ins=[],
                        outs=[],
                        sync_info=mybir.SyncInfo(on_wait=[w], on_update=[]),
                    )
                    ev.engine = ins.engine
                    new_insts.append(ev)
                si.on_wait = waits[-_MAX_WAITS:]
            new_insts.append(ins)
        blk.instructions = new_insts

    for fn in nc.m.functions:
        for blk in fn.blocks:
            fix_block(blk)


def _chunk_plan(rpp: int):
    """Rows-per-partition per chunk. Small chunks at both ends shorten the
    pipeline fill (first compute can't start before chunk 0 lands) and the
    tail (last chunk's compute latency after the final DMA byte)."""
    if rpp == 16384:
        plan = [512, 1024, 1536] + [2048] * 5 + [1536, 1024, 256, 256]
    else:
        # small test sizes: four equal chunks
        assert rpp % 4 == 0
        plan = [rpp // 4] * 4
    assert sum(plan) == rpp and all(f % 256 == 0 for f in plan)
    return plan


def _build(rows_per_core: int):
    """Build the per-core Bass module. All cores run the same program on
    their own shard (pure data parallel, no collectives)."""
    key = rows_per_core
    if key in _cache:
        return _cache[key]

    assert rows_per_core % P == 0
    rpp = rows_per_core // P  # rows per partition
    plan = _chunk_plan(rpp)
    nch = len(plan)
    fmax = max(plan)

    nc = bass.Bass(trn_type="TRN2")
    dtf = mybir.dt.float32
    dti = mybir.dt.int32
    dtb = mybir.dt.bfloat16
    Op = mybir.AluOpType
    Act = mybir.ActivationFunctionType

    x = nc.dram_tensor("x", [P, 2 * rpp], dtf, kind="ExternalInput")
    lab = nc.dram_tensor("lab", [P, rpp], dti, kind="ExternalInput")
    # accumulator columns: [ce | pred | sign(u)] x nch chunks
    acc = nc.dram_tensor("acc", [P, 3 * nch], dtf, kind="ExternalOutput")
    # per-chunk full-tile sum(labels) from the GpSimd partition reduce
    accl = nc.dram_tensor("accl", [1, nch], dtf, kind="ExternalOutput")

    with tile.TileContext(nc) as tc:
        with (
            tc.tile_pool(name="io", bufs=5) as io_pool,
            tc.tile_pool(name="mid", bufs=3) as mid,
            tc.tile_pool(name="junk", bufs=2) as junk,
            tc.tile_pool(name="singles", bufs=1) as singles,
        ):
            st = singles.tile([P, 3 * nch], dtf)
            slt = singles.tile([1, nch], dtf)

            r0 = 0
            for c, F in enumerate(plan):
                r1 = r0 + F
                xt_full = io_pool.tile([P, 2 * fmax], dtf, tag="xt")
                xt = xt_full[:, : 2 * F]
                nc.sync.dma_start(out=xt, in_=x[:, 2 * r0 : 2 * r1])
                xp = xt.rearrange("p (f two) -> p f two", two=2)
                lt_full = io_pool.tile([P, fmax], dti, tag="lt")
                lv = lt_full[:, :F]
                nc.sync.dma_start(out=lv, in_=lab[:, r0:r1])

                # d = x1 - x0
                d_full = mid.tile([P, fmax], dtb, tag="d")
                d = d_full[:, :F]
                nc.vector.tensor_sub(out=d, in0=xp[:, :, 1], in1=xp[:, :, 0])
                # u = (lab - 0.5) * d  (sign-folded margin; ce = softplus(-2u))
                u_full = mid.tile([P, fmax], dtb, tag="u")
                u = u_full[:, :F]
                nc.vector.scalar_tensor_tensor(
                    out=u, in0=lv, scalar=0.5, in1=d,
                    op0=Op.subtract, op1=Op.mult,
                )
                # pred = (d > 0), accumulated per partition
                jp_full = junk.tile([P, fmax], dtb, tag="jp")
                jp = jp_full[:, :F]
                nc.vector.tensor_scalar(
                    out=jp, in0=d, scalar1=0.0, scalar2=None, op0=Op.is_gt,
                    op1=Op.add,
                    accum_out=st[:, nch + c : nch + c + 1],
                )
                # sum(labels) over the whole chunk tile on GpSimd
                nc.gpsimd.tensor_reduce(
                    out=slt[:, c : c + 1], in_=lv,
                    axis=mybir.AxisListType.XYZWC, op=Op.add,
                )
                # CE partial on ACT: t = exp(-2u); softplus = ln(1+t), accum
                t_full = mid.tile([P, fmax], dtb, tag="t")
                t = t_full[:, :F]
                nc.scalar.activation(out=t, in_=u, func=Act.Exp, scale=-2.0)
                ja_full = junk.tile([P, fmax], dtb, tag="ja")
                ja = ja_full[:, :F]
                nc.scalar.activation(
                    out=ja, in_=t, func=Act.Ln, bias=1.0, scale=1.0,
                    accum_out=st[:, c : c + 1],
                )
                # sum(sign(u)) on ACT -> 2*sum(correct) - count
                js_full = junk.tile([P, fmax], dtb, tag="js")
                js = js_full[:, :F]
                nc.scalar.activation(
                    out=js, in_=u, func=Act.Sign,
                    accum_out=st[:, 2 * nch + c : 2 * nch + c + 1],
                )
                r0 = r1

            nc.sync.dma_start(out=acc[:], in_=st)
            nc.sync.dma_start(out=accl[:], in_=slt)

    _cache[key] = (nc, nch)
    return nc, nch


def _combine(acc: np.ndarray, accl: np.ndarray, nch: int, B: int) -> np.ndarray:
    """Host-side scalar epilogue.

    acc: [n_cores, P, 3*nch] f32 per-chunk partial sums, column blocks
    [ce | pred | sign(u)]. accl: [n_cores, 1, nch] f32 label sums.
    Counts are exact integers in fp32 at every stage."""
    a = acc.astype(np.float64).reshape(-1, 3, nch)
    CE, p1, S_u = a.sum(axis=(0, 2))
    n1 = accl.astype(np.float64).sum()  # labels == 1
    C = (S_u + B) / 2.0  # sum(correct); sign(0) rows are negligible
    TP = (C + p1 + n1 - B) / 2.0
    TN = C - TP
    FP = p1 - TP
    FN = n1 - TP

    ce = CE / B
    mean_cs = FN / B
    nonzero = (TP > 0) and (TN > 0) and (FP > 0) and (FN > 0)
    ratio = (TP / max(TP + FN, 1.0)) * (FP / max(FP + TN, 1.0))
    if nonzero:
        coeff = -LAMBD * np.log(np.sqrt(max(ratio, 1e-30)))
    else:
        coeff = LAMBD
    return np.array(ce + coeff * mean_cs, dtype=np.float32)


def run(outputs: np.ndarray, labels: np.ndarray):
    """Run on 8 cores; returns (loss, BassKernelResults)."""
    outputs = np.asarray(outputs)
    labels = np.asarray(labels)
    B = outputs.shape[0]
    assert outputs.shape == (B, 2) and labels.shape == (B,)
    assert B % (N_CORES * P) == 0
    S = B // N_CORES
    rpp = S // P

    if labels.dtype.itemsize == 8:
        # int64: keep the value-bearing little-endian low words
        labels = np.ascontiguousarray(labels.view(np.int32)[::2])
    nc, nch = _build(S)
    _split_multiwaits(nc)  # idempotent; CoreSim needs the unsplit module

    in_maps = []
    for i in range(N_CORES):
        xs = np.ascontiguousarray(outputs[i * S : (i + 1) * S], dtype=np.float32)
        xs = xs.reshape(P, 2 * rpp)
        ls = np.ascontiguousarray(labels[i * S : (i + 1) * S], dtype=np.int32)
        ls = ls.reshape(P, rpp)
        in_maps.append({"x": xs, "lab": ls})

    res = run_bass_kernel_spmd(nc, in_maps, core_ids=list(range(N_CORES)))
    acc = np.stack([r["acc"] for r in res.results])
    accl = np.stack([r["accl"] for r in res.results])
    return _combine(acc, accl, nch, B), res


def kernel(outputs: np.ndarray, labels: np.ndarray) -> np.ndarray:
    return run(outputs, labels)[0]


# revision 21
# speedup vs baseline: 1.2472x; 1.2472x over previous
"""Trainium2 Bass kernel for nn_DetectionLoss (2-class detection loss).

Computes, over B=2^24 rows of logits [B,2] and labels [B]:
  ce    = mean(-log_softmax(outputs)[label])
  pred  = argmax(outputs, axis=1)
  confusion counts TP/TN/FP/FN from (label, pred)
  CS    = M[pred, label] with M = [[0,1],[0,0]]  -> mean(CS) = FN/B
  loss  = ce + coeff(TP,TN,FP,FN) * mean(CS)

Device math (2 classes): with d = x1 - x0 and h = label - 0.5:
  u       = d*h                  # sign-folded logit margin
  ce_row  = softplus(-2u) = log(1 + exp(-2u))
  pred    = (d > 0)
  correct = (u > 0)              # prediction == label
Counts follow from three linear sums (n1 = sum(h) + B/2, p1 = sum(pred),
TP + TN = sum(correct) = C):
  TP = (C + p1 + n1 - B) / 2, TN = C - TP, FP = p1 - TP, FN = n1 - TP.

Engine split per chunk (roughly balanced against the ~7.4 us the two
input DMAs of a 2048-row chunk take):
  DVE: h = lab - 0.5;  d = x1 - x0;  u = d*h;
       pred = (d > 0) with fused accum -> sum(pred)      (~5.8 us)
  ACT: t = exp(-2u); ln(1+t) with accum -> CE partial;
       Sign(u) with accum -> 2*sum(correct) - count      (~6.0 us)
  PE : sum(h) via ones-vector matmuls accumulated in PSUM
       (two banks, alternating per 512-slab)             (~2.5 us)
Inputs stream through SBUF in variable-size chunks (small at both ends
to shorten pipeline fill/drain). The tiny per-core partials are
combined on the host; count arithmetic is exact (half-integers in
fp32 at every stage).

Sharding: data-parallel over the batch dim across 8 NeuronCores.
"""

import numpy as np

import concourse.bass as bass
import concourse.mybir as mybir
import concourse.tile as tile
from concourse.bass_utils import run_bass_kernel_spmd

N_CORES = 8
P = 128
LAMBD = 0.5
MMN = 512  # matmul rhs free-dim tile (one PSUM bank)

_cache = {}

_MAX_WAITS = 1  # this walrus build rejects >1 embedded sync-wait per instruction


def _split_multiwaits(nc):
    """Walrus in this container can't encode instructions with multiple
    sync waits; hoist all but the last into standalone EventSemaphore
    waits on the same engine immediately before the instruction."""
    n = [0]

    def fix_block(blk):
        new_insts = []
        for ins in blk.instructions:
            si = ins.sync_info
            if si is not None and si.on_wait and len(si.on_wait) > _MAX_WAITS:
                waits = list(si.on_wait)
                for w in waits[: -_MAX_WAITS]:
                    n[0] += 1
                    ev = mybir.InstEventSemaphore(
                        name=f"I-waitsplit-{n[0]}",
                        ins=[],
                        outs=[],
                        sync_info=mybir.SyncInfo(on_wait=[w], on_update=[]),
                    )
                    ev.engine = ins.engine
                    new_insts.append(ev)
                si.on_wait = waits[-_MAX_WAITS:]
            new_insts.append(ins)
        blk.instructions = new_insts

    for fn in nc.m.functions:
        for blk in fn.blocks:
            fix_block(blk)


def _chunk_plan(rpp: int):
    """Rows-per-partition per chunk. Small chunks at both ends shorten the
    pipeline fill (first compute can't start before chunk 0 lands) and the
    tail (last chunk's compute latency after the final DMA byte)."""
    if rpp == 16384:
        plan = [512, 1024, 1536] + [2048] * 5 + [1536, 1024, 256, 256]
    else:
        # small test sizes: four equal chunks
        assert rpp % 4 == 0
        plan = [rpp // 4] * 4
    assert sum(plan) == rpp and all(f % 256 == 0 for f in plan)
    return plan


def _build(rows_per_core: int):
    """Build the per-core Bass module. All cores run the same program on
    their own shard (pure data parallel, no collectives)."""
    key = rows_per_core
    if key in _cache:
        return _cache[key]

    assert rows_per_core % P == 0
    rpp = rows_per_core // P  # rows per partition
    plan = _chunk_plan(rpp)
    nch = len(plan)
    fmax = max(plan)

    nc = bass.Bass(trn_type="TRN2")
    dtf = mybir.dt.float32
    dti = mybir.dt.int32
    dtb = mybir.dt.bfloat16
    Op = mybir.AluOpType
    Act = mybir.ActivationFunctionType

    x = nc.dram_tensor("x", [P, 2 * rpp], dtf, kind="ExternalInput")
    lab = nc.dram_tensor("lab", [P, rpp], dti, kind="ExternalInput")
    # accumulator columns: [ce | pred | sign(u)] x nch chunks
    acc = nc.dram_tensor("acc", [P, 3 * nch], dtf, kind="ExternalOutput")
    acc_h = nc.dram_tensor("acc_h", [1, 2 * MMN], dtf, kind="ExternalOutput")

    with tile.TileContext(nc) as tc:
        with (
            tc.tile_pool(name="io", bufs=5) as io_pool,
            tc.tile_pool(name="mid", bufs=2) as mid,
            tc.tile_pool(name="junk", bufs=2) as junk,
            tc.tile_pool(name="singles", bufs=1) as singles,
            tc.tile_pool(name="ps", bufs=1, space="PSUM") as psp,
        ):
            ones = singles.tile([P, 1], dtb)
            nc.vector.memset(ones, 1.0)
            st = singles.tile([P, 3 * nch], dtf)
            ps_h = [
                psp.tile([1, MMN], dtf, tag=f"ps_h{i}", name=f"ps_h{i}")
                for i in range(2)
            ]
            nslab_total = sum((F + MMN - 1) // MMN for F in plan)

            r0 = 0
            kslab = 0
            for c, F in enumerate(plan):
                r1 = r0 + F
                xt_full = io_pool.tile([P, 2 * fmax], dtf, tag="xt")
                xt = xt_full[:, : 2 * F]
                nc.sync.dma_start(out=xt, in_=x[:, 2 * r0 : 2 * r1])
                xp = xt.rearrange("p (f two) -> p f two", two=2)
                lt_full = io_pool.tile([P, fmax], dti, tag="lt")
                lv = lt_full[:, :F]
                nc.sync.dma_start(out=lv, in_=lab[:, r0:r1])

                # h = label - 0.5 in {-0.5,+0.5}
                h_full = mid.tile([P, fmax], dtb, tag="h")
                h = h_full[:, :F]
                nc.vector.tensor_scalar(
                    out=h, in0=lv, scalar1=0.5, scalar2=None, op0=Op.subtract
                )
                # d = x1 - x0
                d_full = mid.tile([P, fmax], dtb, tag="d")
                d = d_full[:, :F]
                nc.vector.tensor_sub(out=d, in0=xp[:, :, 1], in1=xp[:, :, 0])
                # u = d*h  (sign-folded margin; ce_row = softplus(-2u))
                u_full = mid.tile([P, fmax], dtb, tag="u")
                u = u_full[:, :F]
                nc.vector.tensor_mul(out=u, in0=d, in1=h)
                # pred = (d > 0), fused accum -> sum(pred) per partition
                jp_full = junk.tile([P, fmax], dtb, tag="jp")
                jp = jp_full[:, :F]
                nc.vector.tensor_scalar(
                    out=jp, in0=d, scalar1=0.0, scalar2=None, op0=Op.is_gt,
                    op1=Op.add,
                    accum_out=st[:, nch + c : nch + c + 1],
                )

                # CE partial on ACT: t = exp(-2u); ln(1+t), accum
                t_full = mid.tile([P, fmax], dtb, tag="t")
                t = t_full[:, :F]
                nc.scalar.activation(out=t, in_=u, func=Act.Exp, scale=-2.0)
                ja_full = junk.tile([P, fmax], dtb, tag="ja")
                ja = ja_full[:, :F]
                nc.scalar.activation(
                    out=ja, in_=t, func=Act.Ln, bias=1.0, scale=1.0,
                    accum_out=st[:, c : c + 1],
                )
                # sum(sign(u)) on ACT -> 2*sum(correct) - count
                js_full = junk.tile([P, fmax], dtb, tag="js")
                js = js_full[:, :F]
                nc.scalar.activation(
                    out=js, in_=u, func=Act.Sign,
                    accum_out=st[:, 2 * nch + c : 2 * nch + c + 1],
                )

                # sum(h) on PE: ones^T @ h accumulates per-column sums into
                # PSUM across all chunks; banks alternate per 512-slab.
                nslab = (F + MMN - 1) // MMN
                for k in range(nslab):
                    sl = slice(k * MMN, min((k + 1) * MMN, F))
                    w = sl.stop - sl.start
                    bank = kslab % 2
                    first = kslab < 2
                    last = kslab >= nslab_total - 2
                    nc.tensor.matmul(
                        ps_h[bank][:, :w], ones, h[:, sl], start=first, stop=last
                    )
                    kslab += 1
                r0 = r1

            nc.sync.dma_start(out=acc[:], in_=st)
            cnt_sb = singles.tile([1, 2 * MMN], dtf)
            nc.vector.tensor_copy(out=cnt_sb[:, 0 * MMN : 1 * MMN], in_=ps_h[0])
            nc.vector.tensor_copy(out=cnt_sb[:, 1 * MMN : 2 * MMN], in_=ps_h[1])
            nc.sync.dma_start(out=acc_h[:], in_=cnt_sb)

    _cache[key] = (nc, nch)
    return nc, nch


def _combine(acc: np.ndarray, acc_h: np.ndarray, nch: int, B: int) -> np.ndarray:
    """Host-side scalar epilogue.

    acc: [n_cores, P, 3*nch] f32 per-chunk partial sums, column blocks
    [ce | pred | sign(u)]. acc_h: [n_cores, 1, 2*MMN] f32 PE-reduced
    sum(h) partials. Counts are exact (half-)integers in fp32."""
    a = acc.astype(np.float64).reshape(-1, 3, nch)
    CE, p1, S_u = a.sum(axis=(0, 2))
    H1 = acc_h.astype(np.float64).sum()
    n1 = H1 + B / 2.0  # labels == 1
    C = (S_u + B) / 2.0  # sum(correct); sign(0) rows are negligible
    TP = (C + p1 + n1 - B) / 2.0
    TN = C - TP
    FP = p1 - TP
    FN = n1 - TP

    ce = CE / B
    mean_cs = FN / B
    nonzero = (TP > 0) and (TN > 0) and (FP > 0) and (FN > 0)
    ratio = (TP / max(TP + FN, 1.0)) * (FP / max(FP + TN, 1.0))
    if nonzero:
        coeff = -LAMBD * np.log(np.sqrt(max(ratio, 1e-30)))
    else:
        coeff = LAMBD
    return np.array(ce + coeff * mean_cs, dtype=np.float32)


def run(outputs: np.ndarray, labels: np.ndarray):
    """Run on 8 cores; returns (loss, BassKernelResults)."""
    outputs = np.asarray(outputs)
    labels = np.asarray(labels)
    B = outputs.shape[0]
    assert outputs.shape == (B, 2) and labels.shape == (B,)
    assert B % (N_CORES * P) == 0
    S = B // N_CORES
    rpp = S // P

    if labels.dtype.itemsize == 8:
        # int64: keep the value-bearing little-endian low words
        labels = np.ascontiguousarray(labels.view(np.int32)[::2])
    nc, nch = _build(S)
    _split_multiwaits(nc)  # idempotent; CoreSim needs the unsplit module

    in_maps = []
    for i in range(N_CORES):
        xs = np.ascontiguousarray(outputs[i * S : (i + 1) * S], dtype=np.float32)
        xs = xs.reshape(P, 2 * rpp)
        ls = np.ascontiguousarray(labels[i * S : (i + 1) * S], dtype=np.int32)
        ls = ls.reshape(P, rpp)
        in_maps.append({"x": xs, "lab": ls})

    res = run_bass_kernel_spmd(nc, in_maps, core_ids=list(range(N_CORES)))
    acc = np.stack([r["acc"] for r in res.results])
    acc_h = np.stack([r["acc_h"] for r in res.results])
    return _combine(acc, acc_h, nch, B), res


def kernel(outputs: np.ndarray, labels: np.ndarray) -> np.ndarray:
    return run(outputs, labels)[0]


# revision 27
# speedup vs baseline: 1.3953x; 1.1187x over previous
"""Trainium2 Bass kernel for nn_DetectionLoss (2-class detection loss).

Computes, over B=2^24 rows of logits [B,2] and labels [B]:
  ce    = mean(-log_softmax(outputs)[label])
  pred  = argmax(outputs, axis=1)
  confusion counts TP/TN/FP/FN from (label, pred)
  CS    = M[pred, label] with M = [[0,1],[0,0]]  -> mean(CS) = FN/B
  loss  = ce + coeff(TP,TN,FP,FN) * mean(CS)

Device math (2 classes): with d = x1 - x0 and h = label - 0.5:
  u       = d*h                  # sign-folded logit margin
  ce_row  = softplus(-2u) = log(1 + exp(-2u))
  pred    = (d > 0)
  correct = (u > 0)              # prediction == label
Counts follow from three linear sums (n1 = sum(h) + B/2, p1 = sum(pred),
TP + TN = sum(correct) = C):
  TP = (C + p1 + n1 - B) / 2, TN = C - TP, FP = p1 - TP, FN = n1 - TP.

Engine split per chunk (roughly balanced against the ~7.4 us the two
input DMAs of a 2048-row chunk take):
  DVE: h = lab - 0.5;  d = x1 - x0;  u = d*h;  pred = d > 0   (~6.0 us)
  ACT: t = exp(-2u); ln(1+t) with accum -> CE partial;
       Sign(u) with accum -> 2*sum(correct) - count           (~6.0 us)
  PE : sum(h), sum(pred) via ones-vector matmuls accumulated
       in PSUM (two banks each, alternating per 512-slab)     (~5.0 us)
Inputs stream through SBUF in variable-size chunks (small at both ends
to shorten pipeline fill/drain). The tiny per-core partials are
combined on the host; count arithmetic is exact (half-integers in
fp32 at every stage).

Sharding: data-parallel over the batch dim across 8 NeuronCores.
"""

import numpy as np

import concourse.bass as bass
import concourse.mybir as mybir
import concourse.tile as tile
from concourse.bass_utils import run_bass_kernel_spmd

N_CORES = 8
P = 128
LAMBD = 0.5
MMN = 512  # matmul rhs free-dim tile (one PSUM bank)

_cache = {}

_MAX_WAITS = 1  # this walrus build rejects >1 embedded sync-wait per instruction


def _split_multiwaits(nc):
    """Walrus in this container can't encode instructions with multiple
    sync waits; hoist all but the last into standalone EventSemaphore
    waits on the same engine immediately before the instruction."""
    n = [0]

    def fix_block(blk):
        new_insts = []
        for ins in blk.instructions:
            si = ins.sync_info
            if si is not None and si.on_wait and len(si.on_wait) > _MAX_WAITS:
                waits = list(si.on_wait)
                for w in waits[: -_MAX_WAITS]:
                    n[0] += 1
                    ev = mybir.InstEventSemaphore(
                        name=f"I-waitsplit-{n[0]}",
                        ins=[],
                        outs=[],
                        sync_info=mybir.SyncInfo(on_wait=[w], on_update=[]),
                    )
                    ev.engine = ins.engine
                    new_insts.append(ev)
                si.on_wait = waits[-_MAX_WAITS:]
            new_insts.append(ins)
        blk.instructions = new_insts

    for fn in nc.m.functions:
        for blk in fn.blocks:
            fix_block(blk)


def _chunk_plan(rpp: int):
    """Rows-per-partition per chunk. Small chunks at both ends shorten the
    pipeline fill (first compute can't start before chunk 0 lands) and the
    tail (last chunk's compute latency after the final DMA byte)."""
    if rpp == 16384:
        plan = [512, 1024, 1536] + [2048] * 5 + [1536, 1024, 256, 256]
    else:
        # small test sizes: four equal chunks
        assert rpp % 4 == 0
        plan = [rpp // 4] * 4
    assert sum(plan) == rpp and all(f % 256 == 0 for f in plan)
    return plan


def _build(rows_per_core: int):
    """Build the per-core Bass module. All cores run the same program on
    their own shard (pure data parallel, no collectives)."""
    key = rows_per_core
    if key in _cache:
        return _cache[key]

    assert rows_per_core % P == 0
    rpp = rows_per_core // P  # rows per partition
    plan = _chunk_plan(rpp)
    nch = len(plan)
    fmax = max(plan)

    nc = bass.Bass(trn_type="TRN2")
    dtf = mybir.dt.float32
    dti = mybir.dt.int32
    dtb = mybir.dt.bfloat16
    Op = mybir.AluOpType
    Act = mybir.ActivationFunctionType

    x = nc.dram_tensor("x", [P, 2 * rpp], dtf, kind="ExternalInput")
    lab = nc.dram_tensor("lab", [P, rpp], dti, kind="ExternalInput")
    # accumulator columns: [ce | sign(u)] x nch chunks (ACT-written only)
    acc = nc.dram_tensor("acc", [P, 2 * nch], dtf, kind="ExternalOutput")
    # PE-reduced [sum(h) | sum(pred)] partials, 2 banks each
    acc_h = nc.dram_tensor("acc_h", [1, 4 * MMN], dtf, kind="ExternalOutput")

    with tile.TileContext(nc) as tc:
        with (
            tc.tile_pool(name="io", bufs=4) as io_pool,
            tc.tile_pool(name="mid", bufs=3) as mid,
            tc.tile_pool(name="junk", bufs=2) as junk,
            tc.tile_pool(name="singles", bufs=1) as singles,
            tc.tile_pool(name="ps", bufs=1, space="PSUM") as psp,
        ):
            ones = singles.tile([P, 1], dtb)
            nc.vector.memset(ones, 1.0)
            st = singles.tile([P, 2 * nch], dtf)
            ps_h = [
                psp.tile([1, MMN], dtf, tag=f"ps_h{i}", name=f"ps_h{i}")
                for i in range(2)
            ]
            ps_p = [
                psp.tile([1, MMN], dtf, tag=f"ps_p{i}", name=f"ps_p{i}")
                for i in range(2)
            ]
            nslab_total = sum((F + MMN - 1) // MMN for F in plan)

            r0 = 0
            kslab = 0
            for c, F in enumerate(plan):
                r1 = r0 + F
                xt_full = io_pool.tile([P, 2 * fmax], dtf, tag="xt")
                xt = xt_full[:, : 2 * F]
                nc.sync.dma_start(out=xt, in_=x[:, 2 * r0 : 2 * r1])
                xp = xt.rearrange("p (f two) -> p f two", two=2)
                lt_full = io_pool.tile([P, fmax], dti, tag="lt")
                lv = lt_full[:, :F]
                nc.sync.dma_start(out=lv, in_=lab[:, r0:r1])

                # h = label - 0.5 in {-0.5,+0.5}
                h_full = mid.tile([P, fmax], dtb, tag="h")
                h = h_full[:, :F]
                nc.vector.tensor_scalar(
                    out=h, in0=lv, scalar1=0.5, scalar2=None, op0=Op.subtract
                )
                # d = x1 - x0
                d_full = mid.tile([P, fmax], dtb, tag="d")
                d = d_full[:, :F]
                nc.vector.tensor_sub(out=d, in0=xp[:, :, 1], in1=xp[:, :, 0])
                # u = d*h  (sign-folded margin; ce_row = softplus(-2u))
                u_full = mid.tile([P, fmax], dtb, tag="u")
                u = u_full[:, :F]
                nc.vector.tensor_mul(out=u, in0=d, in1=h)
                # pred = (d > 0); summed on the PE below
                jp_full = mid.tile([P, fmax], dtb, tag="jp")
                jp = jp_full[:, :F]
                nc.vector.tensor_scalar(
                    out=jp, in0=d, scalar1=0.0, scalar2=None, op0=Op.is_gt
                )

                # CE partial on ACT: t = exp(-2u); ln(1+t), accum
                t_full = mid.tile([P, fmax], dtb, tag="t")
                t = t_full[:, :F]
                nc.scalar.activation(out=t, in_=u, func=Act.Exp, scale=-2.0)
                ja_full = junk.tile([P, fmax], dtb, tag="ja")
                ja = ja_full[:, :F]
                nc.scalar.activation(
                    out=ja, in_=t, func=Act.Ln, bias=1.0, scale=1.0,
                    accum_out=st[:, c : c + 1],
                )
                # sum(sign(u)) on ACT -> 2*sum(correct) - count
                js_full = junk.tile([P, fmax], dtb, tag="js")
                js = js_full[:, :F]
                nc.scalar.activation(
                    out=js, in_=u, func=Act.Sign,
                    accum_out=st[:, nch + c : nch + c + 1],
                )

                # sum(h), sum(pred) on PE: ones^T @ tile accumulates column
                # sums into PSUM across chunks; banks alternate per slab.
                nslab = (F + MMN - 1) // MMN
                for k in range(nslab):
                    sl = slice(k * MMN, min((k + 1) * MMN, F))
                    w = sl.stop - sl.start
                    bank = kslab % 2
                    first = kslab < 2
                    last = kslab >= nslab_total - 2
                    nc.tensor.matmul(
                        ps_h[bank][:, :w], ones, h[:, sl], start=first, stop=last
                    )
                    nc.tensor.matmul(
                        ps_p[bank][:, :w], ones, jp[:, sl], start=first, stop=last
                    )
                    kslab += 1
                r0 = r1

            nc.sync.dma_start(out=acc[:], in_=st)
            cnt_sb = singles.tile([1, 4 * MMN], dtf)
            nc.vector.tensor_copy(out=cnt_sb[:, 0 * MMN : 1 * MMN], in_=ps_h[0])
            nc.vector.tensor_copy(out=cnt_sb[:, 1 * MMN : 2 * MMN], in_=ps_h[1])
            nc.vector.tensor_copy(out=cnt_sb[:, 2 * MMN : 3 * MMN], in_=ps_p[0])
            nc.vector.tensor_copy(out=cnt_sb[:, 3 * MMN : 4 * MMN], in_=ps_p[1])
            nc.sync.dma_start(out=acc_h[:], in_=cnt_sb)

    _cache[key] = (nc, nch)
    return nc, nch


def _combine(acc: np.ndarray, acc_h: np.ndarray, nch: int, B: int) -> np.ndarray:
    """Host-side scalar epilogue.

    acc: [n_cores, P, 2*nch] f32 per-chunk partial sums, column blocks
    [ce | sign(u)]. acc_h: [n_cores, 1, 4*MMN] f32 PE-reduced
    [sum(h) | sum(pred)] partials. Counts exact (half-)integers in f32."""
    a = acc.astype(np.float64).reshape(-1, 2, nch)
    CE, S_u = a.sum(axis=(0, 2))
    hp = acc_h.astype(np.float64).reshape(-1, 2, 2 * MMN).sum(axis=(0, 2))
    H1, p1 = hp
    n1 = H1 + B / 2.0  # labels == 1
    C = (S_u + B) / 2.0  # sum(correct); sign(0) rows are negligible
    TP = (C + p1 + n1 - B) / 2.0
    TN = C - TP
    FP = p1 - TP
    FN = n1 - TP

    ce = CE / B
    mean_cs = FN / B
    nonzero = (TP > 0) and (TN > 0) and (FP > 0) and (FN > 0)
    ratio = (TP / max(TP + FN, 1.0)) * (FP / max(FP + TN, 1.0))
    if nonzero:
        coeff = -LAMBD * np.log(np.sqrt(max(ratio, 1e-30)))
    else:
        coeff = LAMBD
    return np.array(ce + coeff * mean_cs, dtype=np.float32)


def run(outputs: np.ndarray, labels: np.ndarray):
    """Run on 8 cores; returns (loss, BassKernelResults)."""
    outputs = np.asarray(outputs)
    labels = np.asarray(labels)
    B = outputs.shape[0]
    assert outputs.shape == (B, 2) and labels.shape == (B,)
    assert B % (N_CORES * P) == 0
    S = B // N_CORES
    rpp = S // P

    if labels.dtype.itemsize == 8:
        # int64: keep the value-bearing little-endian low words
        labels = np.ascontiguousarray(labels.view(np.int32)[::2])
    nc, nch = _build(S)
    _split_multiwaits(nc)  # idempotent; CoreSim needs the unsplit module

    in_maps = []
    for i in range(N_CORES):
        xs = np.ascontiguousarray(outputs[i * S : (i + 1) * S], dtype=np.float32)
        xs = xs.reshape(P, 2 * rpp)
        ls = np.ascontiguousarray(labels[i * S : (i + 1) * S], dtype=np.int32)
        ls = ls.reshape(P, rpp)
        in_maps.append({"x": xs, "lab": ls})

    res = run_bass_kernel_spmd(nc, in_maps, core_ids=list(range(N_CORES)))
    acc = np.stack([r["acc"] for r in res.results])
    acc_h = np.stack([r["acc_h"] for r in res.results])
    return _combine(acc, acc_h, nch, B), res


def kernel(outputs: np.ndarray, labels: np.ndarray) -> np.ndarray:
    return run(outputs, labels)[0]
